# revision 1
# baseline (speedup 1.0000x reference)
"""MoE (top-2 routed SwiGLU) kernel for 8 Trainium2 NeuronCores.

Strategy (expert-parallel, host-routed dispatch):
  * Host: router matmul x@Wg.T (+bg), top-k + softmax weights, sort tokens
    by expert, pad each expert's token list to a shared capacity C.
  * Device (SPMD over 8 cores, core e owns expert e):
        OUT_e[D, C] = W2_e @ (silu(W1_e @ X_e) * (W3_e @ X_e))
    All matmuls run as float32r (full PE rate at N>=256, ~1e-4 rel err).
    Weights stream through SBUF once, chunked over the DFF axis; the
    output accumulates in SBUF across DFF chunks.
  * Host: y[tok] += w_tok_e * OUT_e[:, pos].T over the k experts per token.

Only top-k expert work is computed (4x less than the dense reference).
"""

import math
import os
import sys

import numpy as np

for _p in ("/opt/trn_rl_repo", "/opt/pypackages"):
    if _p not in sys.path:
        sys.path.append(_p)

import concourse.bass as bass  # noqa: E402
import concourse.tile as tile  # noqa: E402
from concourse import bacc, bass_utils, mybir  # noqa: E402

F32 = mybir.dt.float32
F32R = mybir.dt.float32r
AF = mybir.ActivationFunctionType

D, DFF, E = 1024, 4096, 8
NCORES = 8
P = 128
KC = D // P            # 8 contraction chunks for the first matmuls
DFFC = 512             # dff columns per weight-stream chunk
NF = DFF // DFFC       # 8 weight-stream iterations
MC2 = DFFC // P        # 4 contraction chunks for the second matmul
C_CAP = 1280           # max padded tokens per expert per device pass

LAST_RESULTS = []      # BassKernelResults per device pass (for test harness)
_NC_CACHE = {}


def _token_blocks(C):
    blocks, t0 = [], 0
    while C - t0 > 512:
        blocks.append((t0, 512))
        t0 += 512
    blocks.append((t0, C - t0))
    return blocks


def _build(C):
    """Compile the per-core expert-FFN program for capacity C."""
    if C in _NC_CACHE:
        return _NC_CACHE[C]
    nc = bacc.Bacc(
        "TRN2", target_bir_lowering=False, debug=False, num_devices=NCORES
    )
    x_d = nc.dram_tensor("xt", [D, C], F32R, kind="ExternalInput")
    w1_d = nc.dram_tensor("w1", [D, DFF], F32R, kind="ExternalInput")
    w3_d = nc.dram_tensor("w3", [D, DFF], F32R, kind="ExternalInput")
    w2_d = nc.dram_tensor("w2", [DFF, D], F32R, kind="ExternalInput")
    o_d = nc.dram_tensor("out", [D, C], F32, kind="ExternalOutput")

    xr = x_d.ap().rearrange("(kc p) c -> p kc c", p=P)
    w1r = w1_d.ap().rearrange("(kc p) f -> p kc f", p=P)
    w3r = w3_d.ap().rearrange("(kc p) f -> p kc f", p=P)
    w2r = w2_d.ap().rearrange("(kc p) d -> p kc d", p=P)
    orr = o_d.ap().rearrange("(mo p) c -> p mo c", p=P)

    tblocks = _token_blocks(C)

    with tile.TileContext(nc) as tc:
        with (
            tc.tile_pool(name="res", bufs=1) as res,
            tc.tile_pool(name="w13", bufs=2) as w13,
            tc.tile_pool(name="w2p", bufs=2) as w2p,
            tc.tile_pool(name="hp", bufs=2) as hp,
            tc.tile_pool(name="sp", bufs=2) as sp,
            tc.tile_pool(name="ps", bufs=2, space="PSUM") as ps,
        ):
            xt = res.tile([P, KC, C], F32R, tag="xt")
            acc = res.tile([P, KC, C], F32, tag="acc")
            for k in range(KC):
                nc.sync.dma_start(xt[:, k, :], xr[:, k, :])

            for fc in range(NF):
                w1t = w13.tile([P, KC, DFFC], F32R, tag="w1")
                w3t = w13.tile([P, KC, DFFC], F32R, tag="w3")
                w2t = w2p.tile([P, MC2, D], F32R, tag="w2")
                fs = fc * DFFC
                for k in range(KC):
                    nc.sync.dma_start(w1t[:, k, :], w1r[:, k, fs:fs + DFFC])
                    nc.sync.dma_start(w3t[:, k, :], w3r[:, k, fs:fs + DFFC])
                for j in range(MC2):
                    nc.sync.dma_start(w2t[:, j, :], w2r[:, fc * MC2 + j, :])

                for (t0, nt) in tblocks:
                    h = hp.tile([P, MC2, 512], F32R, tag="h")
                    for m in range(MC2):
                        ph1 = ps.tile([P, 512], F32, tag="ph1")
                        ph3 = ps.tile([P, 512], F32, tag="ph3")
                        for k in range(KC):
                            nc.tensor.matmul(
                                ph1[:, :nt],
                                w1t[:, k, m * P:(m + 1) * P],
                                xt[:, k, t0:t0 + nt],
                                start=(k == 0),
                                stop=(k == KC - 1),
                            )
                        for k in range(KC):
                            nc.tensor.matmul(
                                ph3[:, :nt],
                                w3t[:, k, m * P:(m + 1) * P],
                                xt[:, k, t0:t0 + nt],
                                start=(k == 0),
                                stop=(k == KC - 1),
                            )
                        s = sp.tile([P, 512], F32, tag="s")
                        nc.scalar.activation(s[:, :nt], ph1[:, :nt], AF.Silu)
                        nc.vector.tensor_mul(
                            h[:, m, :nt], s[:, :nt], ph3[:, :nt]
                        )
                    for mo in range(KC):
                        po = ps.tile([P, 512], F32, tag="po")
                        for j in range(MC2):
                            nc.tensor.matmul(
                                po[:, :nt],
                                w2t[:, j, mo * P:(mo + 1) * P],
                                h[:, j, :nt],
                                start=(j == 0),
                                stop=(j == MC2 - 1),
                            )
                        if fc == 0:
                            nc.scalar.activation(
                                acc[:, mo, t0:t0 + nt], po[:, :nt], AF.Copy
                            )
                        else:
                            nc.vector.tensor_add(
                                acc[:, mo, t0:t0 + nt],
                                acc[:, mo, t0:t0 + nt],
                                po[:, :nt],
                            )

            for mo in range(KC):
                nc.sync.dma_start(orr[:, mo, :], acc[:, mo, :])

    nc.compile()
    _NC_CACHE[C] = nc
    return nc


def kernel(x, Wg, bg, W1, W2, W3, top_k):
    global LAST_RESULTS
    LAST_RESULTS = []
    x = np.ascontiguousarray(np.asarray(x), dtype=np.float32)
    Wg = np.asarray(Wg, dtype=np.float32)
    bg = np.asarray(bg, dtype=np.float32)
    W1 = np.asarray(W1, dtype=np.float32)
    W2 = np.asarray(W2, dtype=np.float32)
    W3 = np.asarray(W3, dtype=np.float32)
    k = int(top_k)
    B, S, D_ = x.shape
    T = B * S
    xt = x.reshape(T, D_)

    # Router (host): logits -> top-k -> softmax over the k selected.
    logits = xt @ Wg.T + bg
    order = np.argsort(-logits, axis=1, kind="stable")
    idx = order[:, :k]                              # [T, k]
    vals = np.take_along_axis(logits, idx, axis=1)
    ex = np.exp(vals - vals.max(axis=1, keepdims=True))
    wts = ex / ex.sum(axis=1, keepdims=True)        # [T, k]

    # Dispatch lists per expert.
    sel, wsel = [], []
    for e in range(E):
        mask = idx == e                             # [T, k]
        rows = np.nonzero(mask.any(axis=1))[0]
        sel.append(rows)
        wsel.append(wts[mask])                      # one weight per row
    max_ne = max(len(s) for s in sel)

    n_pass = max(1, math.ceil(max_ne / C_CAP))
    if n_pass == 1:
        C = min(C_CAP, max(512, 256 * math.ceil(max_ne / 256)))
    else:
        C = C_CAP
    nc = _build(C)

    # Pre-transposed per-expert weights.
    w1t = [np.ascontiguousarray(W1[e].T) for e in range(E)]
    w3t = [np.ascontiguousarray(W3[e].T) for e in range(E)]
    w2t = [np.ascontiguousarray(W2[e].T) for e in range(E)]

    y = np.zeros((T, D_), dtype=np.float32)
    for p_i in range(n_pass):
        in_maps = []
        toks = []
        for e in range(E):
            tok = sel[e][p_i * C:(p_i + 1) * C]
            toks.append(tok)
            XT = np.zeros((D_, C), dtype=np.float32)
            if len(tok):
                XT[:, :len(tok)] = xt[tok].T
            in_maps.append(
                {"xt": XT, "w1": w1t[e], "w3": w3t[e], "w2": w2t[e]}
            )
        res = bass_utils.run_bass_kernel_spmd(
            nc, in_maps, core_ids=list(range(NCORES))
        )
        LAST_RESULTS.append(res)
        for e in range(E):
            tok = toks[e]
            n = len(tok)
            if n == 0:
                continue
            out_e = res.results[e]["out"]           # [D, C]
            w_e = wsel[e][p_i * C:p_i * C + n]
            y[tok] += w_e[:, None] * out_e[:, :n].T

    return y.reshape(B, S, D_)


# revision 2
# speedup vs baseline: 1.1326x; 1.1326x over previous
"""MoE (top-2 routed SwiGLU) kernel for 8 Trainium2 NeuronCores.

Strategy (expert-parallel, host-routed dispatch):
  * Host: router matmul x@Wg.T (+bg), top-k + softmax weights, sort tokens
    by expert, pad each expert's token list to a shared capacity C.
  * Device (SPMD over 8 cores, core e owns expert e):
        OUT_e[D, C] = W2_e @ (silu(W1_e @ X_e) * (W3_e @ X_e))
    All matmuls run as float32r (full PE rate at N>=256, ~1e-4 rel err).
    Weights stream through SBUF once, chunked over the DFF axis; the
    output accumulates in SBUF across DFF chunks.
  * Host: y[tok] += w_tok_e * OUT_e[:, pos].T over the k experts per token.

Only top-k expert work is computed (4x less than the dense reference).
"""

import math
import sys

import numpy as np

for _p in ("/opt/trn_rl_repo", "/opt/pypackages"):
    if _p not in sys.path:
        sys.path.append(_p)

import concourse.bass as bass  # noqa: E402
import concourse.tile as tile  # noqa: E402
from concourse import bacc, bass_utils, mybir  # noqa: E402

F32 = mybir.dt.float32
F32R = mybir.dt.float32r
AF = mybir.ActivationFunctionType

D, DFF, E = 1024, 4096, 8
NCORES = 8
P = 128
KC = D // P            # 8 contraction chunks for the first matmuls
DFFC = 512             # dff columns per weight-stream chunk
NF = DFF // DFFC       # 8 weight-stream iterations
MC2 = DFFC // P        # 4 contraction chunks for the second matmul
C_CAP = 1280           # max padded tokens per expert per device pass

LAST_RESULTS = []      # BassKernelResults per device pass (for test harness)
_NC_CACHE = {}


def _install_ntff_hook():
    """Best-effort: register the axon NTFF profile hook so that
    BASS_TRACE=1 yields exec_time_ns even in a bare environment."""
    try:
        import types
        if "antenv.axon_hooks" not in sys.modules:
            mod = types.ModuleType("antenv.axon_hooks")
            holder = {}
            mod.set_axon_ntff_profile_hook = lambda h: holder.__setitem__("h", h)
            mod.get_axon_ntff_profile_hook = lambda: holder.get("h")
            sys.modules["antenv.axon_hooks"] = mod
            import antenv
            antenv.axon_hooks = mod
        mod = sys.modules["antenv.axon_hooks"]
        if mod.get_axon_ntff_profile_hook() is None:
            from trn_agent_boot.trn_boot import _ntff_profile_via_ctypes
            hook = _ntff_profile_via_ctypes("/opt/axon/libaxon_pjrt.so")
            if hook is not None:
                mod.set_axon_ntff_profile_hook(hook)
    except Exception:
        pass


_install_ntff_hook()


def _token_blocks(C):
    """Split C (multiple of 128, >=256) into blocks of 256..512 columns
    (fp32r matmul needs N>=256 for full PE rate)."""
    blocks, rem, t0 = [], C, 0
    while rem > 640:
        blocks.append((t0, 512))
        t0 += 512
        rem -= 512
    if rem > 512:
        blocks.append((t0, rem - 256))
        blocks.append((t0 + rem - 256, 256))
    else:
        blocks.append((t0, rem))
    return blocks


def _build(C):
    """Compile the per-core expert-FFN program for capacity C."""
    if C in _NC_CACHE:
        return _NC_CACHE[C]
    nc = bacc.Bacc(
        "TRN2", target_bir_lowering=False, debug=False, num_devices=NCORES
    )
    x_d = nc.dram_tensor("xt", [D, C], F32R, kind="ExternalInput")
    w1_d = nc.dram_tensor("w1", [D, DFF], F32R, kind="ExternalInput")
    w3_d = nc.dram_tensor("w3", [D, DFF], F32R, kind="ExternalInput")
    w2_d = nc.dram_tensor("w2", [DFF, D], F32R, kind="ExternalInput")
    o_d = nc.dram_tensor("out", [D, C], F32, kind="ExternalOutput")

    xr = x_d.ap().rearrange("(kc p) c -> p kc c", p=P)
    w1r = w1_d.ap().rearrange("(kc p) f -> p kc f", p=P)
    w3r = w3_d.ap().rearrange("(kc p) f -> p kc f", p=P)
    w2r = w2_d.ap().rearrange("(kc p) d -> p kc d", p=P)
    orr = o_d.ap().rearrange("(mo p) c -> p mo c", p=P)

    tblocks = _token_blocks(C)

    with tile.TileContext(nc) as tc:
        with (
            tc.tile_pool(name="res", bufs=1) as res,
            tc.tile_pool(name="w13", bufs=2) as w13,
            tc.tile_pool(name="w2p", bufs=2) as w2p,
            tc.tile_pool(name="hp", bufs=3) as hp,
            tc.tile_pool(name="sp", bufs=3) as sp,
            tc.tile_pool(name="ps13", bufs=3, space="PSUM") as ps13,
            tc.tile_pool(name="pso", bufs=2, space="PSUM") as pso,
        ):
            xt = res.tile([P, KC, C], F32R, tag="xt")
            acc = res.tile([P, KC, C], F32, tag="acc")

            def load_w13(fc):
                w1t = w13.tile([P, KC, DFFC], F32R, tag="w1")
                w3t = w13.tile([P, KC, DFFC], F32R, tag="w3")
                fs = fc * DFFC
                for k in range(KC):
                    nc.sync.dma_start(w1t[:, k, :], w1r[:, k, fs:fs + DFFC])
                    nc.sync.dma_start(w3t[:, k, :], w3r[:, k, fs:fs + DFFC])
                return w1t, w3t

            def load_w2(fc):
                w2t = w2p.tile([P, MC2, D], F32R, tag="w2")
                for j in range(MC2):
                    nc.sync.dma_start(w2t[:, j, :], w2r[:, fc * MC2 + j, :])
                return w2t

            # Startup order: first token block of x, then fc0 W1/W3 (what
            # the first matmul group needs), then the rest of x, then W2.
            t0_first, nt_first = tblocks[0]
            for k in range(KC):
                nc.sync.dma_start(
                    xt[:, k, t0_first:t0_first + nt_first],
                    xr[:, k, t0_first:t0_first + nt_first],
                )
            w1t0, w3t0 = load_w13(0)
            for (t0, nt) in tblocks[1:]:
                for k in range(KC):
                    nc.sync.dma_start(
                        xt[:, k, t0:t0 + nt], xr[:, k, t0:t0 + nt]
                    )
            w2t0 = load_w2(0)

            for fc in range(NF):
                if fc == 0:
                    w1t, w3t, w2t = w1t0, w3t0, w2t0
                else:
                    w1t, w3t = load_w13(fc)
                    w2t = load_w2(fc)

                for (t0, nt) in tblocks:
                    h = hp.tile([P, MC2, 512], F32R, tag="h")
                    for m in range(MC2):
                        ph1 = ps13.tile([P, 512], F32, tag="ph1")
                        ph3 = ps13.tile([P, 512], F32, tag="ph3")
                        for k in range(KC):
                            nc.tensor.matmul(
                                ph1[:, :nt],
                                w1t[:, k, m * P:(m + 1) * P],
                                xt[:, k, t0:t0 + nt],
                                start=(k == 0),
                                stop=(k == KC - 1),
                            )
                        for k in range(KC):
                            nc.tensor.matmul(
                                ph3[:, :nt],
                                w3t[:, k, m * P:(m + 1) * P],
                                xt[:, k, t0:t0 + nt],
                                start=(k == 0),
                                stop=(k == KC - 1),
                            )
                        s = sp.tile([P, 512], F32, tag="s")
                        nc.scalar.activation(s[:, :nt], ph1[:, :nt], AF.Silu)
                        nc.vector.tensor_mul(
                            h[:, m, :nt], s[:, :nt], ph3[:, :nt]
                        )
                    for mo in range(KC):
                        po = pso.tile([P, 512], F32, tag="po")
                        for j in range(MC2):
                            nc.tensor.matmul(
                                po[:, :nt],
                                w2t[:, j, mo * P:(mo + 1) * P],
                                h[:, j, :nt],
                                start=(j == 0),
                                stop=(j == MC2 - 1),
                            )
                        if fc == 0:
                            nc.scalar.activation(
                                acc[:, mo, t0:t0 + nt], po[:, :nt], AF.Copy
                            )
                        else:
                            nc.vector.tensor_add(
                                acc[:, mo, t0:t0 + nt],
                                acc[:, mo, t0:t0 + nt],
                                po[:, :nt],
                            )
                        if fc == NF - 1:
                            # Stream the finished chunk out while the rest
                            # of the last dff iteration still computes.
                            nc.sync.dma_start(
                                orr[:, mo, t0:t0 + nt],
                                acc[:, mo, t0:t0 + nt],
                            )

    nc.compile()
    _NC_CACHE[C] = nc
    return nc


def kernel(x, Wg, bg, W1, W2, W3, top_k):
    global LAST_RESULTS
    LAST_RESULTS = []
    x = np.ascontiguousarray(np.asarray(x), dtype=np.float32)
    Wg = np.asarray(Wg, dtype=np.float32)
    bg = np.asarray(bg, dtype=np.float32)
    W1 = np.asarray(W1, dtype=np.float32)
    W2 = np.asarray(W2, dtype=np.float32)
    W3 = np.asarray(W3, dtype=np.float32)
    k = int(top_k)
    B, S, D_ = x.shape
    T = B * S
    xt = x.reshape(T, D_)

    # Router (host): logits -> top-k -> softmax over the k selected.
    logits = xt @ Wg.T + bg
    order = np.argsort(-logits, axis=1, kind="stable")
    idx = order[:, :k]                              # [T, k]
    vals = np.take_along_axis(logits, idx, axis=1)
    ex = np.exp(vals - vals.max(axis=1, keepdims=True))
    wts = ex / ex.sum(axis=1, keepdims=True)        # [T, k]

    # Dispatch lists per expert.
    sel, wsel = [], []
    for e in range(E):
        mask = idx == e                             # [T, k]
        rows = np.nonzero(mask.any(axis=1))[0]
        sel.append(rows)
        wsel.append(wts[mask])                      # one weight per row
    max_ne = max(len(s) for s in sel)

    n_pass = max(1, math.ceil(max_ne / C_CAP))
    if n_pass == 1:
        C = min(C_CAP, max(256, P * math.ceil(max_ne / P)))
    else:
        C = C_CAP
    nc = _build(C)

    # Pre-transposed per-expert weights.
    w1t = [np.ascontiguousarray(W1[e].T) for e in range(E)]
    w3t = [np.ascontiguousarray(W3[e].T) for e in range(E)]
    w2t = [np.ascontiguousarray(W2[e].T) for e in range(E)]

    y = np.zeros((T, D_), dtype=np.float32)
    for p_i in range(n_pass):
        in_maps = []
        toks = []
        for e in range(E):
            tok = sel[e][p_i * C:(p_i + 1) * C]
            toks.append(tok)
            XT = np.zeros((D_, C), dtype=np.float32)
            if len(tok):
                XT[:, :len(tok)] = xt[tok].T
            in_maps.append(
                {"xt": XT, "w1": w1t[e], "w3": w3t[e], "w2": w2t[e]}
            )
        res = bass_utils.run_bass_kernel_spmd(
            nc, in_maps, core_ids=list(range(NCORES))
        )
        LAST_RESULTS.append(res)
        for e in range(E):
            tok = toks[e]
            n = len(tok)
            if n == 0:
                continue
            out_e = res.results[e]["out"]           # [D, C]
            w_e = wsel[e][p_i * C:p_i * C + n]
            y[tok] += w_e[:, None] * out_e[:, :n].T

    return y.reshape(B, S, D_)
